# revision 20
# baseline (speedup 1.0000x reference)
"""Trainium2 Bass kernel for nn_Decoder_G (retrieval_knn).

out = MLP(emb1 - knn_interp(emb2, h_pos2, h_pos1))
      + knn_interp(l_y1 - knn_interp(l_y2, l_pos2, l_pos1), l_pos1, h_pos1)

Design (v2, reworked from the dense per-tile-gather baseline):
 * Queries are grouped into spatially-compact 128-query tiles; each tile
   gets an exact-coverage candidate window (host two-pass d3-bound).
   Tiles are assigned to (core, slot) by candidate-count rank so every
   core's slot-j window width is shared (static program) yet close to
   that tile's true need -- scan widths average ~440/200 instead of
   uniform 768/384.
 * Distance scores on the PE at fp32r rate via the 21-row mantissa-split
   trick (exact fp32 scores). Scores stay in PSUM; DVE max8/max_index
   read PSUM directly (no Activation evacuation).
 * All indirect gathers are batched: one SWDGE instruction per 4-16
   tiles (994ns fixed overhead amortized), with multi-column offset APs.
 * Weight math (d2 -> 1/d2 -> normalize) is batched across tiles with
   3D strided APs; interpolation runs in fp16 (DVE 2x/4x modes).
 * MLP in fp16; the 3-wide L3 output is computed query-major so the
   residual add is a single op and the store needs no transpose.
 * ydelta (l_y1 + b3 - small interp) is fp16; b3 is folded in on host
   (interp weights sum to 1), so the MLP bias b3 costs nothing.
"""
import os

import numpy as np

try:  # persistent jax/PJRT executable cache to avoid recompiles across runs
    import jax

    os.makedirs(os.path.expanduser("~/.cache/jax_bass"), exist_ok=True)
    jax.config.update("jax_compilation_cache_dir",
                      os.path.expanduser("~/.cache/jax_bass"))
    jax.config.update("jax_persistent_cache_min_compile_time_secs", 0)
except Exception:
    pass

import concourse.bass as bass
import concourse.mybir as mybir
from concourse import bacc
from concourse.tile import TileContext
from concourse.bass_utils import run_bass_kernel_spmd
from concourse.masks import make_identity

F32 = mybir.dt.float32
F32R = mybir.dt.float32r
F16 = mybir.dt.float16
U32 = mybir.dt.uint32
AF = mybir.ActivationFunctionType
OP = mybir.AluOpType

NCORES = 8
NH, NL, H, O = 16384, 4096, 256, 3
HSH = NH // NCORES      # 2048 h-queries per core
LSH = NL // NCORES      # 512 l-queries per core
NTB = HSH // 128        # 16 big/mid slots per core
NTS = LSH // 128        # 4 small slots per core
FP = 8                  # padded feature width for the 3-wide y-delta
KQ = 21                 # split-fp32r contraction rows
D2_CLIP = 1e-12
DUMMY_NEG = -1048576.0  # scan score of padding entries


# --------------------------------------------------------------------------
# device program
# --------------------------------------------------------------------------

def build_nc(WB, WM, WS, MD):
    """WB/WM/WS: per-slot scan widths (16/16/4); MD: mid dense gather cols."""
    SWB, SWM, SWS = sum(WB), sum(WM), sum(WS)
    cumB = np.concatenate([[0], np.cumsum(WB)]).astype(int)
    cumM = np.concatenate([[0], np.cumsum(WM)]).astype(int)
    cumS = np.concatenate([[0], np.cumsum(WS)]).astype(int)

    # fpack columns: hqn[16] lqn[4] b1[2] b2[2] ly1[4*8]
    C_HQN, C_LQN, C_B1, C_B2, C_LY1 = 0, 16, 20, 22, 24
    FPW = 24 + NTS * FP
    # upack columns: mcand[MD] moff[16] rowB[16] rowS[4]
    C_MC, C_MOFF, C_ROWB, C_ROWS = 0, MD, MD + 16, MD + 32
    UPW = MD + 36
    # wpack columns: W1[2*256] W2[2*256] W3[2*3]
    C_W1, C_W2, C_W3 = 0, 512, 1024
    WPW = 1030

    nc = bacc.Bacc("TRN2", target_bir_lowering=False, debug=False)

    # qpack: [lq (LSH) | small-src (SWS) | hq (HSH)] so one DMA covers
    # everything phase A needs
    qpack = nc.dram_tensor("qpack", [KQ, LSH + SWS + HSH], F32R,
                           kind="ExternalInput")
    bsrc = nc.dram_tensor("bsrc", [KQ, SWB], F32R, kind="ExternalInput")
    msrc = nc.dram_tensor("msrc", [KQ, SWM], F32R, kind="ExternalInput")
    bfeat = nc.dram_tensor("bfeat", [SWB, H], F16, kind="ExternalInput")
    sfeat = nc.dram_tensor("sfeat", [SWS, FP], F32, kind="ExternalInput")
    e1f = nc.dram_tensor("e1f", [H, HSH], F16, kind="ExternalInput")
    wpack = nc.dram_tensor("wpack", [128, WPW], F16, kind="ExternalInput")
    fpack = nc.dram_tensor("fpack", [128, FPW], F32, kind="ExternalInput")
    upack = nc.dram_tensor("upack", [128, UPW], U32, kind="ExternalInput")

    outq = nc.dram_tensor("outq", [128, NTB * O], F32, kind="ExternalOutput")

    with TileContext(nc) as tc:
        with tc.tile_pool(name="p", bufs=1) as pool, \
             tc.tile_pool(name="ps", bufs=1, space="PSUM") as psum_pool, \
             tc.tile_pool(name="dram", bufs=1, space="DRAM") as dram_pool:

            # --- staged constants (phase-A dependencies first) --------------
            qsb = pool.tile([KQ, LSH + SWS + HSH], F32R, name="qsb", tag="qsb")
            nc.sync.dma_start(out=qsb[:, 0:LSH + SWS],
                              in_=qpack[:, 0:LSH + SWS])
            fsb = pool.tile([128, FPW], F32, name="fsb", tag="fsb")
            nc.sync.dma_start(out=fsb[:, :], in_=fpack[:, :])
            usb = pool.tile([128, UPW], U32, name="usb", tag="usb")
            nc.sync.dma_start(out=usb[:, :], in_=upack[:, :])
            nc.sync.dma_start(out=qsb[:, LSH + SWS:],
                              in_=qpack[:, LSH + SWS:])
            bsb = pool.tile([KQ, SWB], F32R, name="bsb", tag="bsb")
            nc.sync.dma_start(out=bsb[:, :], in_=bsrc[:, :])
            msb = pool.tile([KQ, SWM], F32R, name="msb", tag="msb")
            nc.sync.dma_start(out=msb[:, :], in_=msrc[:, :])
            wsb = pool.tile([128, WPW], F16, name="wsb", tag="wsb")
            nc.sync.dma_start(out=wsb[:, :], in_=wpack[:, :])
            e1sb = pool.tile([128, 2, HSH], F16, name="e1sb", tag="e1sb")
            nc.sync.dma_start(out=e1sb[:, :, :],
                              in_=e1f[:, :].rearrange("(h p) q -> p h q", p=128))

            ident = pool.tile([128, 128], F16, name="ident", tag="ident")
            make_identity(nc, ident[:])

            # dram intermediates
            ydelta_sh = dram_pool.tile([LSH, FP], F16, name="ydelta_sh")
            ydelta_full = dram_pool.tile([NL, FP], F16, name="ydelta_full",
                                         addr_space="Shared")
            # p-major dense table: row p*MD+j holds dense slot (p, j)
            ydc_dense = dram_pool.tile([128 * MD, FP], F16, name="ydc_dense")

            # L3 output accumulator (query-major), persistent across phases
            psL3 = psum_pool.tile([128, NTB, O], F32, name="psL3", tag="psL3")

            # --- helpers ---------------------------------------------------
            def scan(qoff, srcsb, soff, w, nm):
                """PE scan -> psum tile holding [128, w] scores."""
                if w <= 512:
                    ps = psum_pool.tile([128, 512], F32, name=f"psA_{nm}",
                                        tag="scanA", bufs=2)
                else:
                    ps = psum_pool.tile([128, 1024], F32, name=f"psB_{nm}",
                                        tag="scanB", bufs=1)
                c0 = min(w, 512)
                nc.tensor.matmul(out=ps[:, 0:c0],
                                 lhsT=qsb[:, qoff:qoff + 128],
                                 rhs=srcsb[:, soff:soff + c0],
                                 start=True, stop=True)
                if w > 512:
                    nc.tensor.matmul(out=ps[:, 512:w],
                                     lhsT=qsb[:, qoff:qoff + 128],
                                     rhs=srcsb[:, soff + 512:soff + w],
                                     start=True, stop=True)
                return ps

            def topk(ps, w, top8, idx8, j, nm):
                nc.vector.max(out=top8[:, j, :], in_=ps[:, 0:w])
                nc.vector.max_index(out=idx8[:, j, :], in_max=top8[:, j, :],
                                    in_values=ps[:, 0:w])

            def wmath(top8, j0, n, qn_off, nm):
                """Batched inverse-d2 weights for slots [j0, j0+n)."""
                d2 = pool.tile([128, n, 3], F32, name=f"d2_{nm}", tag=f"d2{n}",
                               bufs=2)
                nc.vector.tensor_tensor(
                    out=d2[:, :, :],
                    in0=fsb[:, qn_off + j0:qn_off + j0 + n].to_broadcast(
                        [128, n, 3]),
                    in1=top8[:, j0:j0 + n, 0:3], op=OP.subtract)
                nc.vector.tensor_scalar_max(d2[:, :, :], d2[:, :, :], D2_CLIP)
                wv = pool.tile([128, n, 3], F32, name=f"wv_{nm}", tag=f"wv{n}",
                               bufs=2)
                nc.vector.reciprocal(wv[:, :, :], d2[:, :, :])
                ws = pool.tile([128, n], F32, name=f"ws_{nm}", tag=f"ws{n}",
                               bufs=2)
                nc.vector.tensor_reduce(out=ws[:, :], in_=wv[:, :, :],
                                        axis=mybir.AxisListType.X, op=OP.add)
                rs = pool.tile([128, n], F32, name=f"rs_{nm}", tag=f"rs{n}",
                               bufs=2)
                nc.vector.reciprocal(rs[:, :], ws[:, :])
                wn = pool.tile([128, n, 3], F32, name=f"wn_{nm}", tag=f"wn{n}",
                               bufs=2)
                nc.vector.tensor_tensor(out=wn[:, :, :], in0=wv[:, :, :],
                                        in1=rs[:, :].to_broadcast([128, n, 3]),
                                        op=OP.mult)
                return wn

            def idxadj(idx8, j0, n, row_off, nm):
                """Window-local top3 ids + per-slot base -> global rows.
                Stored k-major [128, 3, n] so the gather out AP is 3D."""
                ix = pool.tile([128, 3, n], U32, name=f"ix_{nm}", tag=f"ix{n}",
                               bufs=2)
                nc.vector.tensor_tensor(
                    out=ix[:, :, :].rearrange("p k j -> p j k"),
                    in0=idx8[:, j0:j0 + n, 0:3],
                    in1=usb[:, row_off + j0:row_off + j0 + n].to_broadcast(
                        [128, n, 3]),
                    op=OP.add)
                return ix

            def interp_bcast(g, wn, n, nf, acc_ap, nm, dt, eng=None):
                """acc[:, j, f] = sum_k wn[j, k] * g[:, k*n+j, f] (batched).
                g is [128, 3n, nf] with k-major rows matching idxadj.
                eng=nc.gpsimd offloads the elementwise work to Pool."""
                eng = eng or nc.vector
                eng.tensor_tensor(
                    out=acc_ap, in0=g[:, 0:n, :],
                    in1=wn[:, :, 0:1].to_broadcast([128, n, nf]), op=OP.mult)
                for k in (1, 2):
                    tmp = pool.tile([128, n, nf], dt, name=f"tm_{nm}_{k}",
                                    tag=f"tm_{nm}", bufs=2)
                    eng.tensor_tensor(
                        out=tmp[:, :, :], in0=g[:, k * n:(k + 1) * n, :],
                        in1=wn[:, :, k:k + 1].to_broadcast([128, n, nf]),
                        op=OP.mult)
                    eng.tensor_tensor(out=acc_ap, in0=acc_ap,
                                      in1=tmp[:, :, :], op=OP.add)

            # =================================================================
            # phase A: small knn -> ydelta shard -> AllGather
            # =================================================================
            top8S = pool.tile([128, NTS, 8], F32, name="top8S", tag="top8S")
            idx8S = pool.tile([128, NTS, 8], U32, name="idx8S", tag="idx8S")
            for j in range(NTS):
                ps = scan(j * 128, qsb, LSH + cumS[j], WS[j], f"s{j}")
                topk(ps, WS[j], top8S, idx8S, j, f"s{j}")
            ixS = idxadj(idx8S, 0, NTS, C_ROWS, "s")
            wnS = wmath(top8S, 0, NTS, C_LQN, "s")
            gS = pool.tile([128, 3 * NTS, FP], F32, name="gS", tag="gS")
            for k in range(3):
                for j in range(NTS):
                    nc.gpsimd.indirect_dma_start(
                        out=gS[:, k * NTS + j, :], out_offset=None,
                        in_=sfeat[:],
                        in_offset=bass.IndirectOffsetOnAxis(
                            ap=ixS[:, k, j:j + 1], axis=0))

            def phase_a_tail():
                accS = pool.tile([128, NTS, FP], F32, name="accS", tag="accS")
                interp_bcast(gS, wnS, NTS, FP, accS[:, :, :], "s", F32,
                             eng=nc.gpsimd)
                ydS = pool.tile([128, NTS, FP], F16, name="ydS", tag="ydS")
                nc.gpsimd.tensor_tensor(
                    out=ydS[:, :, :],
                    in0=fsb[:, C_LY1:C_LY1 + NTS * FP].rearrange(
                        "p (j f) -> p j f", f=FP),
                    in1=accS[:, :, :], op=OP.subtract)
                nc.sync.dma_start(
                    out=ydelta_sh[:, :].rearrange("(j p) f -> p j f", p=128),
                    in_=ydS[:, :, :])

            # =================================================================
            # phase B: big knn + fp16 interp + MLP (4-slot pipelined chunks)
            # =================================================================
            top8B = pool.tile([128, NTB, 8], F32, name="top8B", tag="top8B")
            idx8B = pool.tile([128, NTB, 8], U32, name="idx8B", tag="idx8B")
            gB = {}
            wnB = {}

            def big_chunk_prep(c):
                """2-slot gather chunk c (weights come from 4-slot blocks)."""
                j0 = c * 2
                ixB = idxadj(idx8B, j0, 2, C_ROWB, f"b{c}")
                g = pool.tile([128, 6, H], F16, name=f"gB{c}", tag="gB",
                              bufs=3)
                for k in range(3):
                    for jj in range(2):
                        nc.gpsimd.indirect_dma_start(
                            out=g[:, k * 2 + jj, :], out_offset=None,
                            in_=bfeat[:],
                            in_offset=bass.IndirectOffsetOnAxis(
                                ap=ixB[:, k, jj:jj + 1], axis=0))
                gB[c] = g

            def big_tile_tail(t):
                """Feature-major delta for big tile t via diag-weighted
                transpose-matmuls: interpT[f,q] = sum_k (g_k^T diag(w_k)),
                accumulated on the PE; one DVE subtract vs emb1^T."""
                c, jj = t // 2, t % 2
                b, jw = t // 4, t % 4
                dg = pool.tile([128, 3, 128], F16, name=f"dg{t}", tag="dg",
                               bufs=3)
                for k in range(3):
                    nc.vector.tensor_scalar(out=dg[:, k, :], in0=ident[:],
                                            scalar1=wnB[b][:, jw, k:k + 1],
                                            scalar2=None, op0=OP.mult)
                # interpT[f,q] = sum_k g_k^T diag(w_k): NORMAL matmuls
                # (fp32 psum accumulation; transpose-mode can't scale/accum)
                tp = psum_pool.tile([128, 2, 128], F32, name=f"tp{t}",
                                    tag="tpB", bufs=1)
                for hh in range(2):
                    for k in range(3):
                        nc.tensor.matmul(
                            out=tp[:, hh, :],
                            lhsT=gB[c][:, k * 2 + jj,
                                       hh * 128:(hh + 1) * 128],
                            rhs=dg[:, k, :],
                            start=(k == 0), stop=(k == 2))
                dtT = pool.tile([128, 2, 128], F16, name=f"dtT{t}", tag="dtT",
                                bufs=6)
                nc.vector.tensor_tensor(
                    out=dtT[:, :, :],
                    in0=e1sb[:, :, t * 128:(t + 1) * 128],
                    in1=tp[:, :, :], op=OP.subtract)
                return dtT

            def mlp_pair(t0, dtT0, dtT1):
                """fp16 MLP on two tiles; L3 lands query-major in psL3.

                Layer tensors are [128, 256] with cols [i*128:(i+1)*128] =
                queries of pair-tile i; h-layer partition = out-feature
                within mh-half, with the two halves in separate tiles."""
                rhs1 = lambda i, kt: (dtT0, dtT1)[i][:, kt, :]
                h1 = []
                for mh in range(2):
                    psm = psum_pool.tile([128, 256], F32, name=f"p1_{mh}_{t0}",
                                         tag="pscratch", bufs=2)
                    for i in range(2):
                        for kt in range(2):
                            nc.tensor.matmul(
                                out=psm[:, i * 128:(i + 1) * 128],
                                lhsT=wsb[:, C_W1 + kt * 256 + mh * 128:
                                         C_W1 + kt * 256 + mh * 128 + 128],
                                rhs=rhs1(i, kt),
                                start=(kt == 0), stop=(kt == 1))
                    h_ = pool.tile([128, 256], F16, name=f"h1_{mh}_{t0}",
                                   tag=f"h_l1_{mh}", bufs=2)
                    nc.scalar.activation(out=h_[:], in_=psm[:], func=AF.Relu,
                                         bias=fsb[:, C_B1 + mh:C_B1 + mh + 1],
                                         scale=1.0)
                    h1.append(h_)
                h2 = []
                for mh in range(2):
                    psm = psum_pool.tile([128, 256], F32, name=f"p2_{mh}_{t0}",
                                         tag="pscratch", bufs=2)
                    for i in range(2):
                        for kt in range(2):
                            nc.tensor.matmul(
                                out=psm[:, i * 128:(i + 1) * 128],
                                lhsT=wsb[:, C_W2 + kt * 256 + mh * 128:
                                         C_W2 + kt * 256 + mh * 128 + 128],
                                rhs=h1[kt][:, i * 128:(i + 1) * 128],
                                start=(kt == 0), stop=(kt == 1))
                    h_ = pool.tile([128, 256], F16, name=f"h2_{mh}_{t0}",
                                   tag=f"h_l2_{mh}", bufs=2)
                    nc.scalar.activation(out=h_[:], in_=psm[:], func=AF.Relu,
                                         bias=fsb[:, C_B2 + mh:C_B2 + mh + 1],
                                         scale=1.0)
                    h2.append(h_)
                for i, tt in enumerate((t0, t0 + 1)):
                    for mh in range(2):
                        nc.tensor.matmul(
                            out=psL3[:, tt, :],
                            lhsT=h2[mh][:, i * 128:(i + 1) * 128],
                            rhs=wsb[:, C_W3 + mh * 3:C_W3 + mh * 3 + 3],
                            start=(mh == 0), stop=(mh == 1))

            # mid-phase state (scans are woven into the big tail)
            top8M = pool.tile([128, NTB, 8], F32, name="top8M", tag="top8M")
            idx8M = pool.tile([128, NTB, 8], U32, name="idx8M", tag="idx8M")
            gM = {}
            wnM = {}
            accM = pool.tile([128, NTB, FP], F32, name="accM", tag="accM")

            def mid_scan(t):
                ps = psum_pool.tile([128, 512], F32, name=f"pmm{t}",
                                    tag="scanA", bufs=2)
                c0 = WM[t]
                nc.tensor.matmul(out=ps[:, 0:c0],
                                 lhsT=qsb[:, LSH + SWS + t * 128:
                                           LSH + SWS + (t + 1) * 128],
                                 rhs=msb[:, cumM[t]:cumM[t] + c0],
                                 start=True, stop=True)
                topk(ps, WM[t], top8M, idx8M, t, f"m{t}")

            def mid_chunk_prep(c, j0, n):
                wnM[c] = wmath(top8M, j0, n, C_HQN, f"m{c}")
                ixM = idxadj(idx8M, j0, n, C_MOFF, f"m{c}")
                g = pool.tile([128, 3 * n, FP], F16, name=f"gM{c}",
                              tag=f"gM{c}", bufs=1)
                for k in range(3):
                    for j in range(n):
                        nc.gpsimd.indirect_dma_start(
                            out=g[:, k * n + j, :], out_offset=None,
                            in_=ydc_dense[:],
                            in_offset=bass.IndirectOffsetOnAxis(
                                ap=ixM[:, k, j:j + 1], axis=0))
                gM[c] = g

            def big_tail_chunk(c):
                dtT0 = big_tile_tail(2 * c)
                dtT1 = big_tile_tail(2 * c + 1)
                mlp_pair(2 * c, dtT0, dtT1)

            for t in range(NTB):
                ps = scan(LSH + SWS + t * 128, bsb, cumB[t], WB[t], f"b{t}")
                topk(ps, WB[t], top8B, idx8B, t, f"b{t}")
                if t % 4 == 3:
                    b = t // 4
                    wnB[b] = wmath(top8B, b * 4, 4, C_HQN, f"b{b}")
                if t % 2 == 1:
                    if t == 3:
                        # phase-A tail here so big topk 0-3 hides the small
                        # gather latency in the DVE queue
                        phase_a_tail()
                    big_chunk_prep(t // 2)
                    if t == 3:
                        nc.gpsimd.collective_compute(
                            "AllGather", OP.bypass,
                            replica_groups=[list(range(NCORES))],
                            ins=[ydelta_sh.opt()], outs=[ydelta_full.opt()])
                    if t >= 5:
                        big_tail_chunk((t - 5) // 2)
                if t == 11:
                    # dense ydelta table: emit once the collective (done by
                    # now) can't head-block upcoming big gathers
                    ydg = pool.tile([128, MD, FP], F16, name="ydg", tag="ydg")
                    for jc in range(MD):
                        nc.gpsimd.indirect_dma_start(
                            out=ydg[:, jc, :], out_offset=None,
                            in_=ydelta_full[:],
                            in_offset=bass.IndirectOffsetOnAxis(
                                ap=usb[:, C_MC + jc:C_MC + jc + 1], axis=0))
                    nc.sync.dma_start(out=ydc_dense[:, :], in_=ydg[:, :, :])
                if t >= 10:
                    # weave mid scans under the big-phase DVE stream
                    mid_scan(2 * (t - 10))
                    mid_scan(2 * (t - 10) + 1)
                if t == 13:
                    mid_chunk_prep(0, 0, 8)

            # ---- tail: remaining mid scans, then big tails hide the last
            # mid-gather latency ----------------------------------------------
            mid_scan(12)
            mid_scan(13)
            mid_scan(14)
            mid_scan(15)
            mid_chunk_prep(1, 8, 6)
            mid_chunk_prep(2, 14, 2)
            big_tail_chunk(6)
            big_tail_chunk(7)
            interp_bcast(gM[0], wnM[0], 8, FP, accM[:, 0:8, :], "m0", F32,
                         eng=nc.gpsimd)
            interp_bcast(gM[1], wnM[1], 6, FP, accM[:, 8:14, :], "m1", F32,
                         eng=nc.gpsimd)
            interp_bcast(gM[2], wnM[2], 2, FP, accM[:, 14:16, :], "m2", F32,
                         eng=nc.gpsimd)

            # final: x = L3(q-major) + (res + b3)
            outsb = pool.tile([128, NTB, O], F32, name="outsb", tag="outsb")
            nc.vector.tensor_tensor(out=outsb[:, :, :], in0=psL3[:, :, :],
                                    in1=accM[:, :, 0:O], op=OP.add)
            nc.sync.dma_start(out=outq[:, :], in_=outsb[:, :, :])
    nc.compile()
    return nc


_NC_CACHE = {}
_LAST_NC = None


def _get_nc(WB, WM, WS, MD):
    global _LAST_NC
    key = (tuple(WB), tuple(WM), tuple(WS), MD)
    if key not in _NC_CACHE:
        _NC_CACHE[key] = build_nc(list(WB), list(WM), list(WS), MD)
    _LAST_NC = _NC_CACHE[key]
    return _NC_CACHE[key]


# --------------------------------------------------------------------------
# host-side prep
# --------------------------------------------------------------------------

def _rm10(x):
    """Round fp32 to 10 explicit mantissa bits (exact under fp32r)."""
    x = np.ascontiguousarray(x, np.float32)
    u = x.view(np.uint32).astype(np.uint64)
    add = np.uint64(1 << 12)
    u = (u + add) & np.uint64(0xFFFFE000)
    return u.astype(np.uint32).view(np.float32)


def _split3(v64):
    """f64 array -> three 10-bit-mantissa f32 pieces summing to ~2^-33."""
    vh = _rm10(v64.astype(np.float32))
    r = v64 - vh.astype(np.float64)
    vm = _rm10(r.astype(np.float32))
    r = r - vm.astype(np.float64)
    vl = _rm10(r.astype(np.float32))
    return vh, vm, vl


def _q21(pos):
    """[N,3] query pieces -> [21, N] lhsT rows (order matches _s21)."""
    p64 = pos.astype(np.float64)
    yh, ym, yl = _split3(p64.T)            # each [3, N]
    one = np.ones((1, len(pos)), np.float32)
    return np.concatenate([yh, yl, ym, one, yh, ym, one, yh, one], 0)


def _s21(pos):
    """[N,3] source pieces -> [21, N] rhs rows: s = 2 y.x - |x|^2."""
    p64 = pos.astype(np.float64)
    xh, xm, xl = _split3(p64.T)
    n2 = (p64 * p64).sum(1)
    n2h, n2m, n2l = _split3(n2)
    return np.concatenate([
        2.0 * xl, 2.0 * xh, 2.0 * xm, -n2l[None, :],
        2.0 * xm, 2.0 * xh, -n2m[None, :], 2.0 * xh, -n2h[None, :]], 0
    ).astype(np.float32)


def _sort_queries(pos, ncores, ngroups):
    """3-level spatial sort: x-shards -> y-groups -> z-sort. Returns perm."""
    n = len(pos)
    perm = np.argsort(pos[:, 0], kind="stable")
    shard = n // ncores
    out = []
    for c in range(ncores):
        ids = perm[c * shard:(c + 1) * shard]
        ids = ids[np.argsort(pos[ids, 1], kind="stable")]
        gsz = shard // ngroups
        for g in range(ngroups):
            gids = ids[g * gsz:(g + 1) * gsz]
            out.append(gids[np.argsort(pos[gids, 2], kind="stable")])
    return np.concatenate(out)


def _d3_bound(qpos, spos, nx=256):
    """Rigorous upper bound on each query's 3rd-NN distance via the nx
    sources nearest in x (a subset's 3rd-smallest distance >= true d3)."""
    order = np.argsort(spos[:, 0], kind="stable")
    sx = spos[order, 0]
    lo = np.clip(np.searchsorted(sx, qpos[:, 0]) - nx // 2, 0, len(spos) - nx)
    idx = lo[:, None] + np.arange(nx)[None, :]
    cand = spos[order[idx]]
    d2 = ((cand - qpos[:, None, :]) ** 2).sum(-1)
    return np.sqrt(np.partition(d2, 2, axis=1)[:, 2])


def _tile_cands(qpos, spos, tiles, nx=256):
    """Per-tile exact-coverage candidate ids (two-pass d3 bound)."""
    d3 = _d3_bound(qpos, spos, nx)
    cands = []
    for ids in tiles:
        r = d3[ids].max()
        bmin = qpos[ids].min(0) - r
        bmax = qpos[ids].max(0) + r
        cand = np.where(((spos >= bmin) & (spos <= bmax)).all(1))[0]
        d2 = ((spos[cand][None, :, :] - qpos[ids][:, None, :]) ** 2).sum(-1)
        r2 = np.sqrt(np.partition(d2, 2, axis=1)[:, 2].max())
        bmin = qpos[ids].min(0) - r2
        bmax = qpos[ids].max(0) + r2
        cand = np.where(((spos >= bmin) & (spos <= bmax)).all(1))[0]
        cands.append(cand)
    return cands


def _roundw(n, lo=64):
    """Scan width for a candidate count: mult of 64, no 192 (fp32r rate)."""
    w = max(lo, 64 * int(np.ceil(n / 64)))
    if w == 192:
        w = 256
    return w


def _balance(counts, nslots):
    """Assign len(counts) tiles to (core, slot) by count rank.
    Returns assign[c][j] = tile index, widths[j]."""
    order = np.argsort(np.asarray(counts), kind="stable")  # ascending
    assign = [[0] * nslots for _ in range(NCORES)]
    widths = []
    for j in range(nslots):
        blk = order[j * NCORES:(j + 1) * NCORES]
        for c in range(NCORES):
            assign[c][j] = int(blk[c])
        widths.append(_roundw(max(counts[t] for t in blk)))
    return assign, widths


_PREP_CACHE = {}


def _prep(h_pos1, l_pos1, h_pos2, l_pos2):
    key = (h_pos1.tobytes(), h_pos2.tobytes(), l_pos1.tobytes(),
           l_pos2.tobytes())
    if _PREP_CACHE.get("k") == key:
        return _PREP_CACHE
    _PREP_CACHE.clear()
    P = _PREP_CACHE
    P["k"] = key
    h1 = np.asarray(h_pos1, np.float32)
    l1 = np.asarray(l_pos1, np.float32)
    h2 = np.asarray(h_pos2, np.float32)
    l2 = np.asarray(l_pos2, np.float32)

    # ---- small phase: 32 l-query tiles, balanced into 4 slots ----
    permL0 = _sort_queries(l1, NCORES, 1)
    ltiles0 = [permL0[i * 128:(i + 1) * 128] for i in range(NCORES * NTS)]
    cs0 = _tile_cands(l1, l2, ltiles0)
    asgS, WS = _balance([len(c) for c in cs0], NTS)
    ltiles = [[ltiles0[asgS[c][j]] for j in range(NTS)] for c in range(NCORES)]
    csS = [[cs0[asgS[c][j]] for j in range(NTS)] for c in range(NCORES)]
    permL = np.concatenate([t for c in range(NCORES) for t in ltiles[c]])
    l1s = l1[permL]

    # ---- big/mid: 128 h-query tiles, balanced into 16 slots ----
    permH0 = _sort_queries(h1, NCORES, 4)
    htiles0 = [permH0[i * 128:(i + 1) * 128] for i in range(NCORES * NTB)]
    cb0 = _tile_cands(h1, h2, htiles0)
    cm0 = _tile_cands(h1, l1s, htiles0)
    cost = [_roundw(len(b)) + _roundw(len(m), lo=64)
            for b, m in zip(cb0, cm0)]
    asgB, _ = _balance(cost, NTB)   # rank by combined width
    WB = [_roundw(max(len(cb0[asgB[c][j]]) for c in range(NCORES)))
          for j in range(NTB)]
    WM = [_roundw(max(len(cm0[asgB[c][j]]) for c in range(NCORES)))
          for j in range(NTB)]
    htiles = [[htiles0[asgB[c][j]] for j in range(NTB)] for c in range(NCORES)]
    cbB = [[cb0[asgB[c][j]] for j in range(NTB)] for c in range(NCORES)]
    cmB = [[cm0[asgB[c][j]] for j in range(NTB)] for c in range(NCORES)]
    permH = np.concatenate([t for c in range(NCORES) for t in htiles[c]])

    assert all(w <= 1024 for w in WB + WM + WS), (WB, WM, WS)
    MD = max(int(np.ceil(sum(len(x) for x in cmB[c]) / 128))
             for c in range(NCORES))

    P.update(permL=permL, permH=permH, l1s=l1s,
             htiles=htiles, ltiles=ltiles, cbB=cbB, cmB=cmB, csS=csS,
             WB=WB, WM=WM, WS=WS, MD=MD,
             q21h=_q21(h1), q21l=_q21(l1s),
             s21b=_s21(h2), s21m=_s21(l1s), s21s=_s21(l2),
             qnh=(h1.astype(np.float64) ** 2).sum(1).astype(np.float32),
             qnl=(l1s.astype(np.float64) ** 2).sum(1).astype(np.float32))
    return P


def _pack_src(s21, cands, widths, total):
    """[21, total] candidate source pieces at per-slot offsets, padded."""
    out = np.zeros((KQ, total), np.float32)
    out[KQ - 1, :] = DUMMY_NEG
    off = 0
    for cand, w in zip(cands, widths):
        out[:, off:off + len(cand)] = s21[:, cand]
        off += w
    return out


def _in_maps(emb1, l_y1, l_pos1, h_pos1, emb2, l_y2, l_pos2, h_pos2,
             W1, b1, W2, b2, W3, b3):
    P = _prep(np.asarray(h_pos1, np.float32), np.asarray(l_pos1, np.float32),
              np.asarray(h_pos2, np.float32), np.asarray(l_pos2, np.float32))
    WB, WM, WS, MD = P["WB"], P["WM"], P["WS"], P["MD"]
    cumB = np.concatenate([[0], np.cumsum(WB)]).astype(np.uint32)
    cumM = np.concatenate([[0], np.cumsum(WM)]).astype(np.uint32)
    cumS = np.concatenate([[0], np.cumsum(WS)]).astype(np.uint32)

    emb1 = np.asarray(emb1, np.float32)
    emb2_16 = np.asarray(emb2, np.float16)
    l_y2 = np.asarray(l_y2, np.float32)
    l_y1b = np.asarray(l_y1, np.float32).copy()
    l_y1b[:, :] += np.asarray(b3, np.float32)[None, :]   # fold b3 into ydelta
    W1h = np.asarray(W1, np.float16)
    W2h = np.asarray(W2, np.float16)
    W3h = np.asarray(W3, np.float16)

    wpack = np.zeros((128, 1030), np.float16)
    for kt in range(2):
        wpack[:, kt * 256:(kt + 1) * 256] = W1h[kt * 128:(kt + 1) * 128, :]
        wpack[:, 512 + kt * 256:512 + (kt + 1) * 256] = \
            W2h[kt * 128:(kt + 1) * 128, :]
        wpack[:, 1024 + kt * 3:1024 + (kt + 1) * 3] = \
            W3h[kt * 128:(kt + 1) * 128, :]

    FPW = 24 + NTS * FP
    UPW = MD + 36

    in_maps = []
    for c in range(NCORES):
        hsl = np.concatenate(P["htiles"][c])       # 2048 query ids, slot order
        lsl = np.concatenate(P["ltiles"][c])       # 512 l-query ids
        cb, cm, cs = P["cbB"][c], P["cmB"][c], P["csS"][c]

        qpack = np.concatenate(
            [P["q21l"][:, c * LSH:(c + 1) * LSH],
             _pack_src(P["s21s"], cs, WS, int(cumS[-1])),
             P["q21h"][:, hsl]], axis=1)

        # feature tables
        bfeat = np.zeros((int(cumB[-1]), H), np.float16)
        for j in range(NTB):
            bfeat[int(cumB[j]):int(cumB[j]) + len(cb[j])] = emb2_16[cb[j]]
        sfeat = np.zeros((int(cumS[-1]), FP), np.float32)
        for j in range(NTS):
            sfeat[int(cumS[j]):int(cumS[j]) + len(cs[j]), :O] = l_y2[cs[j]]

        # mid dense candidate table
        dense = np.concatenate(cm) if len(cm) else np.zeros(0, np.int64)
        assert len(dense) <= MD * 128
        starts = np.concatenate(
            [[0], np.cumsum([len(x) for x in cm])])[:NTB].astype(np.uint32)
        unw = np.zeros(MD * 128, np.uint32)
        unw[:len(dense)] = dense
        # p-major dense layout: dense slot d lives at ydc row d = p*MD + j
        mcand = np.ascontiguousarray(unw.reshape(128, MD))

        fpack = np.zeros((128, FPW), np.float32)
        fpack[:, 0:16] = P["qnh"][hsl].reshape(NTB, 128).T
        fpack[:, 16:20] = P["qnl"][c * LSH:(c + 1) * LSH].reshape(NTS, 128).T
        fpack[:, 20:22] = np.asarray(b1, np.float32).reshape(2, 128).T
        fpack[:, 22:24] = np.asarray(b2, np.float32).reshape(2, 128).T
        ly = np.zeros((128, NTS, FP), np.float32)
        for j in range(NTS):
            ly[:, j, :O] = l_y1b[P["ltiles"][c][j]]
        fpack[:, 24:24 + NTS * FP] = ly.reshape(128, NTS * FP)

        upack = np.zeros((128, UPW), np.uint32)
        upack[:, 0:MD] = mcand
        upack[:, MD:MD + 16] = np.broadcast_to(starts[None, :], (128, NTB))
        upack[:, MD + 16:MD + 32] = np.broadcast_to(cumB[None, :NTB],
                                                    (128, NTB))
        upack[:, MD + 32:MD + 36] = np.broadcast_to(cumS[None, :NTS],
                                                    (128, NTS))

        m = dict(
            qpack=np.ascontiguousarray(qpack),
            bsrc=_pack_src(P["s21b"], cb, WB, int(cumB[-1])),
            msrc=_pack_src(P["s21m"], cm, WM, int(cumM[-1])),
            bfeat=bfeat, sfeat=sfeat,
            e1f=np.ascontiguousarray(emb1[hsl].T.astype(np.float16)),
            wpack=wpack,
            fpack=np.ascontiguousarray(fpack),
            upack=np.ascontiguousarray(upack),
        )
        in_maps.append(m)
    return in_maps


def kernel(**inputs):
    in_maps = _in_maps(**inputs)
    P = _PREP_CACHE
    nc = _get_nc(P["WB"], P["WM"], P["WS"], P["MD"])
    res = run_bass_kernel_spmd(nc, in_maps, list(range(NCORES)))
    permH = P["permH"]
    out = np.empty((NH, O), np.float32)
    for c in range(NCORES):
        oq = res.results[c]["outq"].reshape(128, NTB, O)
        out[permH[c * HSH:(c + 1) * HSH], :] = (
            oq.transpose(1, 0, 2).reshape(HSH, O))
    return out


def run_traced(inputs):
    in_maps = _in_maps(**inputs)
    P = _PREP_CACHE
    nc = _get_nc(P["WB"], P["WM"], P["WS"], P["MD"])
    return run_bass_kernel_spmd(nc, in_maps, list(range(NCORES)), trace=True)
